# revision 2
# baseline (speedup 1.0000x reference)
"""ARAP loss kernel for Trainium2 (8 NeuronCores, SPMD).

Strategy (walk-stream, v3):
  - The reference's unique directed edge list is mirror-symmetric, so
    only j<k edges are processed and the sum is doubled (exact).
  - The undirected edge multiset is decomposed into trails (walks) by
    pairing up edge-incidences at every vertex; the concatenated walk
    visits E + #trails vertices and every adjacent pair of visits is
    exactly one edge.  The host materializes per-visit vertex records
    (p=x+dx, q=x-dx for all 16 batches, bf16, 192B) in walk order,
    split into 8 per-core streams of 128 partition-runs laid out
    [128, L, 96]; consecutive runs overlap by one visit.
  - The device does NO gathers: it streams the records with big
    sequential DMAs (wide ~120-pair tiles, single HWDGE queue - a
    second queue measured slower) and computes per adjacent column pair
        d = rec[:, c+1, :] - rec[:, c, :]
        m = d[:, :, 0:48] * d[:, :, 48:96]       (= dp * dq)
        s = m[c0] + m[c1] + m[c2]                (per batch)
    all on DVE.  Measured rates: DVE 0.514 ns/col (2x bf16); Pool
    1.82 ns/col AND any Pool/DVE concurrency measured far below the
    sum of rates (SBUF contention), so the whole chain stays on DVE.
  - ACT writes |s| into per-tile slots of a wide bf16 buffer (4 sA
    buffers so DVE never stalls on ACT); one pairwise tree reduction
    per pass collapses the slots, then acc[128,16] f32.
  - Trail-junction pairs (one per trail) are junk; the host computes
    their exact contribution from the same bf16-rounded records and
    subtracts it.  Padding repeats the previous record so pad pairs
    are exactly zero.
  - Host sums [128,16] partials over partitions/cores, scales by 2/E.

Per-vertex records hold p = x+dx and q = x-dx (a linear recoding done
once per vertex):  diffx - diffdx = sum_c (pk-pj)*(qk-qj)  exactly.
Record field layout (columns of the 96-wide record):
    f = h*48 + c*16 + b   (h: 0=p/1=q, c: xyz, b: batch)
"""

import sys

sys.path.insert(0, "/opt/trn_rl_repo")

import numpy as np
import ml_dtypes

import concourse.bass as bass
import concourse.tile as tile
from concourse import bacc, mybir
from concourse.bass_utils import run_bass_kernel_spmd

NV = 100000
B = 16
N_CORES = 8
NRUN = N_CORES * 128       # total partition-runs

_nc_cache = {}


def _make_widths(L, chunk=120, head=(16, 32), tail=(32, 16)):
    pairs = L - 1
    body = pairs - sum(head) - sum(tail)
    assert body > 0
    n_body = body // chunk
    rem = body - n_body * chunk
    plist = list(head) + [chunk] * n_body + ([rem] if rem else []) + list(tail)
    return tuple(p + 1 for p in plist)


def _build_nc(params, repeat=1):
    """params = (widths, L) from _prepare; widths are re-derived with
    wide (120-pair) body tiles.  repeat>1 wraps the pass in a hardware
    loop (timing only)."""
    _, L = params
    key = (L, repeat)
    if key in _nc_cache:
        return _nc_cache[key]

    bf16 = mybir.dt.bfloat16
    f32 = mybir.dt.float32

    widths = _make_widths(L)
    assert sum(w - 1 for w in widths) == L - 1
    n_tiles = len(widths)
    starts = [0]
    for wd in widths:
        starts.append(starts[-1] + wd - 1)
    slotA = [0]
    for wd in widths:
        slotA.append(slotA[-1] + wd - 1)
    PA = slotA[-1]
    CAmax = max(w - 1 for w in widths)

    nc = bacc.Bacc("TRN2", target_bir_lowering=False, debug=False,
                   num_devices=N_CORES)
    stream_ap = nc.dram_tensor("stream", [128, L, 96], bf16,
                               kind="ExternalInput").ap()
    out_ap = nc.dram_tensor("out", [128, 16], f32, kind="ExternalOutput").ap()

    with tile.TileContext(nc) as tc:
        with tc.tile_pool(name="acc", bufs=1) as acc_pool, \
             tc.tile_pool(name="g", bufs=3) as g_pool, \
             tc.tile_pool(name="cmp", bufs=2) as cmp_pool:

            acc = acc_pool.tile([128, 16], f32)
            nc.vector.memset(acc[:], 0.0)
            absA = acc_pool.tile([128, PA, 16], bf16)

            with tc.For_i(0, repeat) as _i:
                for t in range(n_tiles):
                    W = widths[t]
                    CA = W - 1
                    g = g_pool.tile([128, max(widths), 96], bf16, tag="g")
                    nc.sync.dma_start(
                        g[:, 0:W, :],
                        stream_ap[:, starts[t]: starts[t] + W, :])

                    dA = cmp_pool.tile([128, CAmax, 96], bf16, tag="dA")
                    nc.vector.tensor_sub(dA[:, 0:CA, :],
                                         g[:, 1:CA + 1, :], g[:, 0:CA, :])
                    mA = cmp_pool.tile([128, CAmax, 48], bf16, tag="mA")
                    nc.vector.tensor_mul(mA[:, 0:CA, :],
                                         dA[:, 0:CA, 0:48],
                                         dA[:, 0:CA, 48:96])
                    sA = cmp_pool.tile([128, CAmax, 16], bf16, tag="sA",
                                       bufs=4)
                    nc.vector.tensor_add(sA[:, 0:CA, :],
                                         mA[:, 0:CA, 0:16],
                                         mA[:, 0:CA, 16:32])
                    nc.vector.tensor_add(sA[:, 0:CA, :], sA[:, 0:CA, :],
                                         mA[:, 0:CA, 32:48])
                    nc.scalar.activation(
                        absA[:, slotA[t]: slotA[t] + CA, :],
                        sA[:, 0:CA, :],
                        mybir.ActivationFunctionType.Abs)

                # pairwise tree over the PA slots (DVE, 2x bf16)
                n = PA
                while n > 1:
                    h = n // 2
                    nc.vector.tensor_add(absA[:, 0:h, :], absA[:, 0:h, :],
                                         absA[:, n - h:n, :])
                    n -= h
                nc.vector.tensor_add(acc[:], acc[:], absA[:, 0, :])

            nc.sync.dma_start(out_ap[:], acc[:])

    nc.finalize()
    _nc_cache[key] = nc
    return nc


def _pack_recs(dx, x):
    recs = np.empty((NV, 2, 3, B), dtype=np.float32)
    recs[:, 0, :, :] = (x + dx).transpose(1, 2, 0)
    recs[:, 1, :, :] = (x - dx).transpose(1, 2, 0)
    return recs.reshape(NV, 96).astype(ml_dtypes.bfloat16)


def _walk(u, w):
    """Trail decomposition of the undirected multigraph {(u_i, w_i)}.
    Returns (ids, break_starts): concatenated visit streams and the
    stream index where each trail starts."""
    E = u.shape[0]
    EP = np.empty(2 * E, dtype=np.int64)
    EP[0::2] = u
    EP[1::2] = w
    order = np.argsort(EP, kind="stable")
    grp_start = np.flatnonzero(np.diff(EP[order], prepend=-1))
    sizes = np.diff(np.append(grp_start, 2 * E))
    P = np.full(2 * E, -1, dtype=np.int64)
    wi = np.arange(2 * E) - np.repeat(grp_start, sizes)
    even = (wi % 2 == 0) & (wi + 1 < np.repeat(sizes, sizes))
    ev = order[even]
    od = order[np.flatnonzero(even) + 1]
    P[ev] = od
    P[od] = ev

    Pl = P.tolist()
    EPl = EP.tolist()
    visited = bytearray(E)
    ids = []
    breaks = []

    def follow(start):
        breaks.append(len(ids))
        i = start
        ids.append(EPl[i])
        while True:
            e = i >> 1
            if visited[e]:
                break
            visited[e] = 1
            j = i ^ 1
            ids.append(EPl[j])
            i = Pl[j]
            if i == -1:
                break

    for s in range(2 * E):
        if Pl[s] == -1 and not visited[s >> 1]:
            follow(s)
    for s in range(2 * E):
        if not visited[s >> 1]:
            follow(s)
    return np.array(ids, dtype=np.int64), np.array(breaks, dtype=np.int64)


def _prepare(dx, x, edges):
    dx = np.asarray(dx, dtype=np.float32)
    x = np.asarray(x, dtype=np.float32)
    edges = np.asarray(edges)
    E = edges.shape[0]
    recs = _pack_recs(dx, x)

    ej = edges[:, 0].astype(np.int64)
    ek = edges[:, 1].astype(np.int64)
    fwd, bwd = ej < ek, ej > ek
    if np.array_equal(np.sort(ej[fwd] * NV + ek[fwd]),
                      np.sort(ek[bwd] * NV + ej[bwd])):
        u, w = ej[fwd], ek[fwd]
        scale = 2.0
    else:
        keep = ej != ek
        u, w = ej[keep], ek[keep]
        scale = 1.0

    ids, breaks = _walk(u, w)
    Ltot = ids.shape[0]

    # runs of run_len visits, consecutive runs overlap by one visit
    run_len = -(-(Ltot - 1) // NRUN) + 1
    pairs = run_len - 1
    # L must leave room for head/tail tiles in _make_widths
    L = max(pairs, 100) + 1
    ids_pad = np.concatenate(
        [ids, np.full(NRUN * (run_len - 1) + 1 - Ltot, ids[-1],
                      dtype=np.int64)])
    runs = ids_pad[(np.arange(NRUN) * (run_len - 1))[:, None]
                   + np.arange(run_len)[None, :]]
    runs = np.concatenate(
        [runs, np.repeat(runs[:, -1:], L - run_len, axis=1)], axis=1)

    # junk: the pair (ids[b-1], ids[b]) preceding each trail start
    jb = breaks[breaks > 0]
    ja, jc = ids[jb - 1], ids[jb]
    ra = recs[ja].astype(np.float64)
    rb = recs[jc].astype(np.float64)
    dd = (ra - rb).reshape(-1, 2, 3, B)
    pq = (dd[:, 0, :, :] * dd[:, 1, :, :]).sum(axis=1)    # [J, B]
    junk = np.abs(pq).sum(axis=0)                         # [B]

    in_maps = []
    for c in range(N_CORES):
        core_ids = runs[c * 128:(c + 1) * 128]       # [128, L]
        in_maps.append({"stream": np.ascontiguousarray(recs[core_ids])})
    return (None, L), in_maps, E, scale, junk


def kernel(dx, x, edges):
    params, in_maps, E, scale, junk = _prepare(dx, x, edges)
    nc = _build_nc(params)
    res = run_bass_kernel_spmd(nc, in_maps, list(range(N_CORES)))
    total = np.zeros(16, dtype=np.float64)
    for c in range(N_CORES):
        total += res.results[c]["out"].astype(np.float64).sum(axis=0)
    return (scale * (total - junk) / E).astype(np.float32)


# revision 4
# speedup vs baseline: 1.0991x; 1.0991x over previous
"""ARAP loss kernel for Trainium2 (8 NeuronCores, SPMD).

Strategy (walk-stream, v3):
  - The reference's unique directed edge list is mirror-symmetric, so
    only j<k edges are processed and the sum is doubled (exact).
  - The undirected edge multiset is decomposed into trails (walks) by
    pairing up edge-incidences at every vertex; the concatenated walk
    visits E + #trails vertices and every adjacent pair of visits is
    exactly one edge.  The host materializes per-visit vertex records
    (p=x+dx, q=x-dx for all 16 batches, bf16, 192B) in walk order,
    split into 8 per-core streams of 128 partition-runs laid out
    [128, L, 96]; consecutive runs overlap by one visit.
  - The device does NO gathers: it streams the records with big
    sequential DMAs (wide ~120-pair tiles, single HWDGE queue - a
    second queue measured slower) and computes per adjacent column pair
        d = rec[:, c+1, :] - rec[:, c, :]
        m = d[:, :, 0:48] * d[:, :, 48:96]       (= dp * dq)
        s = m[c0] + m[c1] + m[c2]                (per batch)
    all on DVE.  Measured rates: DVE 0.514 ns/col (2x bf16); Pool
    1.82 ns/col AND any Pool/DVE concurrency measured far below the
    sum of rates (SBUF contention), so the whole chain stays on DVE.
  - ACT writes |s| into per-tile slots of a wide bf16 buffer (4 sA
    buffers so DVE never stalls on ACT); one pairwise tree reduction
    per pass collapses the slots, then acc[128,16] f32.
  - Trail-junction pairs (one per trail) are junk; the host computes
    their exact contribution from the same bf16-rounded records and
    subtracts it.  Padding repeats the previous record so pad pairs
    are exactly zero.
  - Host sums [128,16] partials over partitions/cores, scales by 2/E.

Per-vertex records hold p = x+dx and q = x-dx (a linear recoding done
once per vertex):  diffx - diffdx = sum_c (pk-pj)*(qk-qj)  exactly.
Record field layout (columns of the 96-wide record):
    f = h*48 + c*16 + b   (h: 0=p/1=q, c: xyz, b: batch)
"""

import sys

sys.path.insert(0, "/opt/trn_rl_repo")

import numpy as np
import ml_dtypes

import concourse.bass as bass
import concourse.tile as tile
from concourse import bacc, mybir
from concourse.bass_utils import run_bass_kernel_spmd

NV = 100000
B = 16
N_CORES = 8
NRUN = N_CORES * 128       # total partition-runs

_nc_cache = {}


def _make_widths(L, chunk=120, head=(16, 32), tail=(32, 16)):
    pairs = L - 1
    body = pairs - sum(head) - sum(tail)
    assert body > 0
    n_body = body // chunk
    rem = body - n_body * chunk
    plist = list(head) + [chunk] * n_body + ([rem] if rem else []) + list(tail)
    return tuple(p + 1 for p in plist)


def _build_nc(params, repeat=1, unroll=1):
    """params = (widths, L) from _prepare; widths are re-derived with
    wide (120-pair) body tiles.  repeat>1 wraps the pass in a hardware
    loop and unroll emits several passes per loop iteration (timing
    only; production uses repeat=unroll=1)."""
    _, L = params
    key = (L, repeat, unroll)
    if key in _nc_cache:
        return _nc_cache[key]

    bf16 = mybir.dt.bfloat16
    f32 = mybir.dt.float32

    widths = _make_widths(L)
    assert sum(w - 1 for w in widths) == L - 1
    n_tiles = len(widths)
    starts = [0]
    for wd in widths:
        starts.append(starts[-1] + wd - 1)
    slotA = [0]
    for wd in widths:
        slotA.append(slotA[-1] + wd - 1)
    PA = slotA[-1]
    CAmax = max(w - 1 for w in widths)

    nc = bacc.Bacc("TRN2", target_bir_lowering=False, debug=False,
                   num_devices=N_CORES)
    stream_ap = nc.dram_tensor("stream", [128, L, 96], bf16,
                               kind="ExternalInput").ap()
    out_ap = nc.dram_tensor("out", [128, 16], f32, kind="ExternalOutput").ap()

    with tile.TileContext(nc) as tc:
        with tc.tile_pool(name="acc", bufs=1) as acc_pool, \
             tc.tile_pool(name="g", bufs=3) as g_pool, \
             tc.tile_pool(name="cmp", bufs=2) as cmp_pool:

            acc = acc_pool.tile([128, 16], f32)
            nc.vector.memset(acc[:], 0.0)
            abs_bufs = [acc_pool.tile([128, PA, 16], bf16,
                                      name="absA%d" % i)
                        for i in range(min(unroll, 2))]

            with tc.For_i(0, repeat) as _i:
                for u in range(unroll):
                    absA = abs_bufs[u % len(abs_bufs)]
                    for t in range(n_tiles):
                        W = widths[t]
                        CA = W - 1
                        g = g_pool.tile([128, max(widths), 96], bf16, tag="g")
                        nc.sync.dma_start(
                            g[:, 0:W, :],
                            stream_ap[:, starts[t]: starts[t] + W, :])

                        dA = cmp_pool.tile([128, CAmax, 96], bf16, tag="dA")
                        nc.vector.tensor_sub(dA[:, 0:CA, :],
                                             g[:, 1:CA + 1, :], g[:, 0:CA, :])
                        mA = cmp_pool.tile([128, CAmax, 48], bf16, tag="mA")
                        nc.vector.tensor_mul(mA[:, 0:CA, :],
                                             dA[:, 0:CA, 0:48],
                                             dA[:, 0:CA, 48:96])
                        sA = cmp_pool.tile([128, CAmax, 16], bf16, tag="sA",
                                           bufs=4)
                        nc.vector.tensor_add(sA[:, 0:CA, :],
                                             mA[:, 0:CA, 0:16],
                                             mA[:, 0:CA, 16:32])
                        nc.vector.tensor_add(sA[:, 0:CA, :], sA[:, 0:CA, :],
                                             mA[:, 0:CA, 32:48])
                        nc.scalar.activation(
                            absA[:, slotA[t]: slotA[t] + CA, :],
                            sA[:, 0:CA, :],
                            mybir.ActivationFunctionType.Abs)

                    # pairwise tree over the PA slots (DVE, 2x bf16)
                    n = PA
                    while n > 1:
                        h = n // 2
                        nc.vector.tensor_add(absA[:, 0:h, :],
                                             absA[:, 0:h, :],
                                             absA[:, n - h:n, :])
                        n -= h
                    nc.vector.tensor_add(acc[:], acc[:], absA[:, 0, :])

            nc.sync.dma_start(out_ap[:], acc[:])

    nc.finalize()
    _nc_cache[key] = nc
    return nc


def _pack_recs(dx, x):
    recs = np.empty((NV, 2, 3, B), dtype=np.float32)
    recs[:, 0, :, :] = (x + dx).transpose(1, 2, 0)
    recs[:, 1, :, :] = (x - dx).transpose(1, 2, 0)
    return recs.reshape(NV, 96).astype(ml_dtypes.bfloat16)


def _walk(u, w):
    """Trail decomposition of the undirected multigraph {(u_i, w_i)}.
    Returns (ids, break_starts): concatenated visit streams and the
    stream index where each trail starts."""
    E = u.shape[0]
    EP = np.empty(2 * E, dtype=np.int64)
    EP[0::2] = u
    EP[1::2] = w
    order = np.argsort(EP, kind="stable")
    grp_start = np.flatnonzero(np.diff(EP[order], prepend=-1))
    sizes = np.diff(np.append(grp_start, 2 * E))
    P = np.full(2 * E, -1, dtype=np.int64)
    wi = np.arange(2 * E) - np.repeat(grp_start, sizes)
    even = (wi % 2 == 0) & (wi + 1 < np.repeat(sizes, sizes))
    ev = order[even]
    od = order[np.flatnonzero(even) + 1]
    P[ev] = od
    P[od] = ev

    Pl = P.tolist()
    EPl = EP.tolist()
    visited = bytearray(E)
    ids = []
    breaks = []

    def follow(start):
        breaks.append(len(ids))
        i = start
        ids.append(EPl[i])
        while True:
            e = i >> 1
            if visited[e]:
                break
            visited[e] = 1
            j = i ^ 1
            ids.append(EPl[j])
            i = Pl[j]
            if i == -1:
                break

    for s in range(2 * E):
        if Pl[s] == -1 and not visited[s >> 1]:
            follow(s)
    for s in range(2 * E):
        if not visited[s >> 1]:
            follow(s)
    return np.array(ids, dtype=np.int64), np.array(breaks, dtype=np.int64)


def _prepare(dx, x, edges):
    dx = np.asarray(dx, dtype=np.float32)
    x = np.asarray(x, dtype=np.float32)
    edges = np.asarray(edges)
    E = edges.shape[0]
    recs = _pack_recs(dx, x)

    ej = edges[:, 0].astype(np.int64)
    ek = edges[:, 1].astype(np.int64)
    fwd, bwd = ej < ek, ej > ek
    if np.array_equal(np.sort(ej[fwd] * NV + ek[fwd]),
                      np.sort(ek[bwd] * NV + ej[bwd])):
        u, w = ej[fwd], ek[fwd]
        scale = 2.0
    else:
        keep = ej != ek
        u, w = ej[keep], ek[keep]
        scale = 1.0

    ids, breaks = _walk(u, w)
    Ltot = ids.shape[0]

    # runs of run_len visits, consecutive runs overlap by one visit
    run_len = -(-(Ltot - 1) // NRUN) + 1
    pairs = run_len - 1
    # L must leave room for head/tail tiles in _make_widths
    L = max(pairs, 100) + 1
    ids_pad = np.concatenate(
        [ids, np.full(NRUN * (run_len - 1) + 1 - Ltot, ids[-1],
                      dtype=np.int64)])
    runs = ids_pad[(np.arange(NRUN) * (run_len - 1))[:, None]
                   + np.arange(run_len)[None, :]]
    runs = np.concatenate(
        [runs, np.repeat(runs[:, -1:], L - run_len, axis=1)], axis=1)

    # junk: the pair (ids[b-1], ids[b]) preceding each trail start
    jb = breaks[breaks > 0]
    ja, jc = ids[jb - 1], ids[jb]
    ra = recs[ja].astype(np.float64)
    rb = recs[jc].astype(np.float64)
    dd = (ra - rb).reshape(-1, 2, 3, B)
    pq = (dd[:, 0, :, :] * dd[:, 1, :, :]).sum(axis=1)    # [J, B]
    junk = np.abs(pq).sum(axis=0)                         # [B]

    in_maps = []
    for c in range(N_CORES):
        core_ids = runs[c * 128:(c + 1) * 128]       # [128, L]
        in_maps.append({"stream": np.ascontiguousarray(recs[core_ids])})
    return (None, L), in_maps, E, scale, junk


def kernel(dx, x, edges):
    params, in_maps, E, scale, junk = _prepare(dx, x, edges)
    nc = _build_nc(params)
    res = run_bass_kernel_spmd(nc, in_maps, list(range(N_CORES)))
    total = np.zeros(16, dtype=np.float64)
    for c in range(N_CORES):
        total += res.results[c]["out"].astype(np.float64).sum(axis=0)
    return (scale * (total - junk) / E).astype(np.float32)


# revision 6
# speedup vs baseline: 1.1564x; 1.0522x over previous
"""ARAP loss kernel for Trainium2 (8 NeuronCores, SPMD).

Strategy (walk-stream, v3):
  - The reference's unique directed edge list is mirror-symmetric, so
    only j<k edges are processed and the sum is doubled (exact).
  - The undirected edge multiset is decomposed into trails (walks) by
    pairing up edge-incidences at every vertex; the concatenated walk
    visits E + #trails vertices and every adjacent pair of visits is
    exactly one edge.  The host materializes per-visit vertex records
    (p=x+dx, q=x-dx for all 16 batches, bf16, 192B) in walk order,
    split into 8 per-core streams of 128 partition-runs laid out
    [128, L, 96]; consecutive runs overlap by one visit.
  - The device does NO gathers: it streams the records with big
    sequential DMAs (wide ~120-pair tiles, single HWDGE queue - a
    second queue measured slower) and computes per adjacent column pair
        d = rec[:, c+1, :] - rec[:, c, :]
        m = d[:, :, 0:48] * d[:, :, 48:96]       (= dp * dq)
        s = m[c0] + m[c1] + m[c2]                (per batch)
    all on DVE.  Measured rates: DVE 0.514 ns/col (2x bf16); Pool
    1.82 ns/col AND any Pool/DVE concurrency measured far below the
    sum of rates (SBUF contention), so the whole chain stays on DVE.
  - ACT writes |s| into per-tile slots of a wide bf16 buffer (4 sA
    buffers so DVE never stalls on ACT); one pairwise tree reduction
    per pass collapses the slots, then acc[128,16] f32.
  - Trail-junction pairs (one per trail) are junk; the host computes
    their exact contribution from the same bf16-rounded records and
    subtracts it.  Padding repeats the previous record so pad pairs
    are exactly zero.
  - Host sums [128,16] partials over partitions/cores, scales by 2/E.

Per-vertex records hold p = x+dx and q = x-dx (a linear recoding done
once per vertex):  diffx - diffdx = sum_c (pk-pj)*(qk-qj)  exactly.
Record field layout (columns of the 96-wide record):
    f = h*48 + c*16 + b   (h: 0=p/1=q, c: xyz, b: batch)
"""

import sys

sys.path.insert(0, "/opt/trn_rl_repo")

import numpy as np
import ml_dtypes

import concourse.bass as bass
import concourse.tile as tile
from concourse import bacc, mybir
from concourse.bass_utils import run_bass_kernel_spmd

NV = 100000
B = 16
N_CORES = 8
NRUN = N_CORES * 128       # total partition-runs

_nc_cache = {}


def _make_widths(L, chunk=120, head=(16, 32), tail=(32, 16)):
    pairs = L - 1
    body = pairs - sum(head) - sum(tail)
    assert body > 0
    n_body = body // chunk
    rem = body - n_body * chunk
    plist = list(head) + [chunk] * n_body + ([rem] if rem else []) + list(tail)
    return tuple(p + 1 for p in plist)


def _build_nc(params, repeat=1, unroll=1, tree_pool=False):
    """params = (widths, L) from _prepare; widths are re-derived with
    wide (120-pair) body tiles.  repeat>1 wraps the pass in a hardware
    loop and unroll emits several passes per loop iteration (timing
    only; production uses repeat=unroll=1).  tree_pool runs the slot
    reduction on the Pool engine so it overlaps the next pass's DVE."""
    _, L = params
    key = (L, repeat, unroll, tree_pool)
    if key in _nc_cache:
        return _nc_cache[key]

    bf16 = mybir.dt.bfloat16
    f32 = mybir.dt.float32

    widths = _make_widths(L)
    assert sum(w - 1 for w in widths) == L - 1
    n_tiles = len(widths)
    starts = [0]
    for wd in widths:
        starts.append(starts[-1] + wd - 1)
    slotA = [0]
    for wd in widths:
        slotA.append(slotA[-1] + wd - 1)
    PA = slotA[-1]
    CAmax = max(w - 1 for w in widths)

    nc = bacc.Bacc("TRN2", target_bir_lowering=False, debug=False,
                   num_devices=N_CORES)
    stream_ap = nc.dram_tensor("stream", [128, L, 96], bf16,
                               kind="ExternalInput").ap()
    out_ap = nc.dram_tensor("out", [128, 16], f32, kind="ExternalOutput").ap()

    with tile.TileContext(nc) as tc:
        with tc.tile_pool(name="acc", bufs=1) as acc_pool, \
             tc.tile_pool(name="g", bufs=3) as g_pool, \
             tc.tile_pool(name="cmp", bufs=2) as cmp_pool:

            acc = acc_pool.tile([128, 16], f32)
            nc.vector.memset(acc[:], 0.0)
            abs_bufs = [acc_pool.tile([128, PA, 16], bf16,
                                      name="absA%d" % i)
                        for i in range(min(unroll, 2))]

            with tc.For_i(0, repeat) as _i:
                for u in range(unroll):
                    absA = abs_bufs[u % len(abs_bufs)]
                    for t in range(n_tiles):
                        W = widths[t]
                        CA = W - 1
                        g = g_pool.tile([128, max(widths), 96], bf16, tag="g")
                        nc.sync.dma_start(
                            g[:, 0:W, :],
                            stream_ap[:, starts[t]: starts[t] + W, :])

                        dA = cmp_pool.tile([128, CAmax, 96], bf16, tag="dA")
                        nc.vector.tensor_sub(dA[:, 0:CA, :],
                                             g[:, 1:CA + 1, :], g[:, 0:CA, :])
                        mA = cmp_pool.tile([128, CAmax, 48], bf16, tag="mA")
                        nc.vector.tensor_mul(mA[:, 0:CA, :],
                                             dA[:, 0:CA, 0:48],
                                             dA[:, 0:CA, 48:96])
                        sA = cmp_pool.tile([128, CAmax, 16], bf16, tag="sA",
                                           bufs=4)
                        nc.vector.tensor_add(sA[:, 0:CA, :],
                                             mA[:, 0:CA, 0:16],
                                             mA[:, 0:CA, 16:32])
                        nc.vector.tensor_add(sA[:, 0:CA, :], sA[:, 0:CA, :],
                                             mA[:, 0:CA, 32:48])
                        nc.scalar.activation(
                            absA[:, slotA[t]: slotA[t] + CA, :],
                            sA[:, 0:CA, :],
                            mybir.ActivationFunctionType.Abs)

                    # pairwise tree over the PA slots (2x bf16)
                    teng = nc.gpsimd if tree_pool else nc.vector
                    n = PA
                    while n > 1:
                        h = n // 2
                        teng.tensor_add(absA[:, 0:h, :],
                                        absA[:, 0:h, :],
                                        absA[:, n - h:n, :])
                        n -= h
                    teng.tensor_add(acc[:], acc[:], absA[:, 0, :])

            nc.sync.dma_start(out_ap[:], acc[:])

    nc.finalize()
    _nc_cache[key] = nc
    return nc


def _pack_recs(dx, x):
    recs = np.empty((NV, 2, 3, B), dtype=np.float32)
    recs[:, 0, :, :] = (x + dx).transpose(1, 2, 0)
    recs[:, 1, :, :] = (x - dx).transpose(1, 2, 0)
    return recs.reshape(NV, 96).astype(ml_dtypes.bfloat16)


def _walk(u, w):
    """Trail decomposition of the undirected multigraph {(u_i, w_i)}.
    Returns (ids, break_starts): concatenated visit streams and the
    stream index where each trail starts."""
    E = u.shape[0]
    EP = np.empty(2 * E, dtype=np.int64)
    EP[0::2] = u
    EP[1::2] = w
    order = np.argsort(EP, kind="stable")
    grp_start = np.flatnonzero(np.diff(EP[order], prepend=-1))
    sizes = np.diff(np.append(grp_start, 2 * E))
    P = np.full(2 * E, -1, dtype=np.int64)
    wi = np.arange(2 * E) - np.repeat(grp_start, sizes)
    even = (wi % 2 == 0) & (wi + 1 < np.repeat(sizes, sizes))
    ev = order[even]
    od = order[np.flatnonzero(even) + 1]
    P[ev] = od
    P[od] = ev

    Pl = P.tolist()
    EPl = EP.tolist()
    visited = bytearray(E)
    ids = []
    breaks = []

    def follow(start):
        breaks.append(len(ids))
        i = start
        ids.append(EPl[i])
        while True:
            e = i >> 1
            if visited[e]:
                break
            visited[e] = 1
            j = i ^ 1
            ids.append(EPl[j])
            i = Pl[j]
            if i == -1:
                break

    for s in range(2 * E):
        if Pl[s] == -1 and not visited[s >> 1]:
            follow(s)
    for s in range(2 * E):
        if not visited[s >> 1]:
            follow(s)
    return np.array(ids, dtype=np.int64), np.array(breaks, dtype=np.int64)


def _prepare(dx, x, edges):
    dx = np.asarray(dx, dtype=np.float32)
    x = np.asarray(x, dtype=np.float32)
    edges = np.asarray(edges)
    E = edges.shape[0]
    recs = _pack_recs(dx, x)

    ej = edges[:, 0].astype(np.int64)
    ek = edges[:, 1].astype(np.int64)
    fwd, bwd = ej < ek, ej > ek
    if np.array_equal(np.sort(ej[fwd] * NV + ek[fwd]),
                      np.sort(ek[bwd] * NV + ej[bwd])):
        u, w = ej[fwd], ek[fwd]
        scale = 2.0
    else:
        keep = ej != ek
        u, w = ej[keep], ek[keep]
        scale = 1.0

    ids, breaks = _walk(u, w)
    Ltot = ids.shape[0]

    # runs of run_len visits, consecutive runs overlap by one visit
    run_len = -(-(Ltot - 1) // NRUN) + 1
    pairs = run_len - 1
    # L must leave room for head/tail tiles in _make_widths
    L = max(pairs, 100) + 1
    ids_pad = np.concatenate(
        [ids, np.full(NRUN * (run_len - 1) + 1 - Ltot, ids[-1],
                      dtype=np.int64)])
    runs = ids_pad[(np.arange(NRUN) * (run_len - 1))[:, None]
                   + np.arange(run_len)[None, :]]
    runs = np.concatenate(
        [runs, np.repeat(runs[:, -1:], L - run_len, axis=1)], axis=1)

    # junk: the pair (ids[b-1], ids[b]) preceding each trail start
    jb = breaks[breaks > 0]
    ja, jc = ids[jb - 1], ids[jb]
    ra = recs[ja].astype(np.float64)
    rb = recs[jc].astype(np.float64)
    dd = (ra - rb).reshape(-1, 2, 3, B)
    pq = (dd[:, 0, :, :] * dd[:, 1, :, :]).sum(axis=1)    # [J, B]
    junk = np.abs(pq).sum(axis=0)                         # [B]

    in_maps = []
    for c in range(N_CORES):
        core_ids = runs[c * 128:(c + 1) * 128]       # [128, L]
        in_maps.append({"stream": np.ascontiguousarray(recs[core_ids])})
    return (None, L), in_maps, E, scale, junk


def kernel(dx, x, edges):
    params, in_maps, E, scale, junk = _prepare(dx, x, edges)
    nc = _build_nc(params)
    res = run_bass_kernel_spmd(nc, in_maps, list(range(N_CORES)))
    total = np.zeros(16, dtype=np.float64)
    for c in range(N_CORES):
        total += res.results[c]["out"].astype(np.float64).sum(axis=0)
    return (scale * (total - junk) / E).astype(np.float32)
